# revision 1
# baseline (speedup 1.0000x reference)
"""Distributed attention-with-2D-relative-position kernel for one TRN2 chip.

Strategy: pure data-parallel over batch. B=64 splits as 8 batches per
NeuronCore across the 8 cores; weights and the tiny RPE tables are
replicated. No collectives are needed — each core computes its batch
shard end-to-end and the host concatenates the shards.

Hardcoded problem shape (nn_AutoformerSpace_67894843015798):
  x (64, 197, 640), Wq/Wk/Wv/Wproj (640, 640), bproj (640,),
  tab_* (30, 64). H=10 heads, head_dim=64.
"""
import numpy as np
import jax
import jax.numpy as jnp

NUM_HEADS = 10
HEAD_DIM = 64
RPE_LEN = 14
N_TOK = 197
N_CORES = 8


def _rpe_indices(N=N_TOK, length=RPE_LEN):
    # Static (input-independent) 2D relative-position index grids.
    Lq = N - 1
    s = int(Lq ** 0.5)
    r = np.arange(Lq)
    dv = r[None, :] // s - r[:, None] // s
    dh = r[None, :] % s - r[:, None] % s
    iv = np.clip(dv, -length, length) + length + 1
    ih = np.clip(dh, -length, length) + length + 1
    iv = np.pad(iv, ((1, 0), (1, 0)))  # cls row/col -> index 0
    ih = np.pad(ih, ((1, 0), (1, 0)))
    return iv.astype(np.int32), ih.astype(np.int32)


_IV, _IH = _rpe_indices()
_PMAPPED = None


def _build():
    global _PMAPPED
    if _PMAPPED is not None:
        return _PMAPPED

    iv = jnp.asarray(_IV)
    ih = jnp.asarray(_IH)

    def shard_fn(x, Wq, Wk, Wv, Wproj, bproj, tab_k_v, tab_k_h, tab_v_v, tab_v_h):
        B, N, E = x.shape
        H, hd = NUM_HEADS, HEAD_DIM
        scale = hd ** -0.5

        q = (x @ Wq).reshape(B, N, H, hd).transpose(0, 2, 1, 3)
        k = (x @ Wk).reshape(B, N, H, hd).transpose(0, 2, 1, 3)
        v = (x @ Wv).reshape(B, N, H, hd).transpose(0, 2, 1, 3)

        attn = jnp.einsum("bhqd,bhkd->bhqk", q, k) * scale
        r_p_k = tab_k_v[iv] + tab_k_h[ih]  # (N, N, hd)
        attn = attn + jnp.einsum("bhqd,qkd->bhqk", q, r_p_k) * scale
        attn = jax.nn.softmax(attn, axis=-1)

        out = jnp.einsum("bhqk,bhkd->bqhd", attn, v)
        r_p_v = tab_v_v[iv] + tab_v_h[ih]
        out = out + jnp.einsum("bhqk,qkd->bqhd", attn, r_p_v)

        out = out.reshape(B, N, H * hd)
        return out @ Wproj + bproj

    _PMAPPED = jax.pmap(
        shard_fn,
        in_axes=(0, None, None, None, None, None, None, None, None, None),
    )
    return _PMAPPED


def kernel(x, Wq, Wk, Wv, Wproj, bproj, tab_k_v, tab_k_h, tab_v_v, tab_v_h):
    f = _build()
    x = np.asarray(x, dtype=np.float32)
    B, N, E = x.shape
    xs = x.reshape(N_CORES, B // N_CORES, N, E)
    out = f(
        xs,
        jnp.asarray(Wq), jnp.asarray(Wk), jnp.asarray(Wv),
        jnp.asarray(Wproj), jnp.asarray(bproj),
        jnp.asarray(tab_k_v), jnp.asarray(tab_k_h),
        jnp.asarray(tab_v_v), jnp.asarray(tab_v_h),
    )
    return np.asarray(out).reshape(B, N, E).astype(np.float32)


# revision 2
# speedup vs baseline: 3.0910x; 3.0910x over previous
"""Distributed attention-with-2D-relative-position kernel for one TRN2 chip.

Strategy: pure data-parallel over batch. B=64 splits as 8 batches per
NeuronCore across the 8 cores; weights and the tiny RPE tables are
replicated. No collectives are needed — each core computes its batch
shard end-to-end and the host concatenates the shards.

Hardcoded problem shape (nn_AutoformerSpace_67894843015798):
  x (64, 197, 640), Wq/Wk/Wv/Wproj (640, 640), bproj (640,),
  tab_* (30, 64). H=10 heads, head_dim=64.
"""
import numpy as np
import jax
import jax.numpy as jnp

NUM_HEADS = 10
HEAD_DIM = 64
RPE_LEN = 14
N_TOK = 197
N_CORES = 8


def _rpe_indices(N=N_TOK, length=RPE_LEN):
    # Static (input-independent) 2D relative-position index grids.
    Lq = N - 1
    s = int(Lq ** 0.5)
    r = np.arange(Lq)
    dv = r[None, :] // s - r[:, None] // s
    dh = r[None, :] % s - r[:, None] % s
    iv = np.clip(dv, -length, length) + length + 1
    ih = np.clip(dh, -length, length) + length + 1
    iv = np.pad(iv, ((1, 0), (1, 0)))  # cls row/col -> index 0
    ih = np.pad(ih, ((1, 0), (1, 0)))
    return iv.astype(np.int32), ih.astype(np.int32)


_IV, _IH = _rpe_indices()
_PMAPPED = None


def _build():
    global _PMAPPED
    if _PMAPPED is not None:
        return _PMAPPED

    # Static one-hot matrices: r_p tables are built with two small matmuls
    # ((N*N, 30) @ (30, hd)) instead of 2.5M-element device gathers.
    ohv = jnp.asarray(np.eye(30, dtype=np.float32)[_IV.ravel()])
    ohh = jnp.asarray(np.eye(30, dtype=np.float32)[_IH.ravel()])

    def shard_fn(x, Wq, Wk, Wv, Wproj, bproj, tab_k_v, tab_k_h, tab_v_v, tab_v_h):
        B, N, E = x.shape
        H, hd = NUM_HEADS, HEAD_DIM
        P = B * H
        scale = hd ** -0.5

        q = (x @ Wq).reshape(B, N, H, hd).transpose(0, 2, 1, 3)
        k = (x @ Wk).reshape(B, N, H, hd).transpose(0, 2, 1, 3)
        v = (x @ Wv).reshape(B, N, H, hd).transpose(0, 2, 1, 3)

        attn = jnp.einsum("bhqd,bhkd->bhqk", q, k)

        r_p_k = (ohv @ tab_k_v + ohh @ tab_k_h).reshape(N, N, hd)
        # per-q bias as one clean batched matmul: (N, P, hd) @ (N, hd, N)
        q_t = q.transpose(2, 0, 1, 3).reshape(N, P, hd)
        bias = jax.lax.batch_matmul(q_t, r_p_k.transpose(0, 2, 1))  # (N, P, N)
        attn = (attn + bias.reshape(N, B, H, N).transpose(1, 2, 0, 3)) * scale
        attn = jax.nn.softmax(attn, axis=-1)

        out = jnp.einsum("bhqk,bhkd->bqhd", attn, v)

        r_p_v = (ohv @ tab_v_v + ohh @ tab_v_h).reshape(N, N, hd)
        a_t = attn.transpose(2, 0, 1, 3).reshape(N, P, N)
        out_r = jax.lax.batch_matmul(a_t, r_p_v)  # (N, P, hd)
        out = out + out_r.reshape(N, B, H, hd).transpose(1, 0, 2, 3)

        out = out.reshape(B, N, H * hd)
        return out @ Wproj + bproj

    _PMAPPED = jax.pmap(
        shard_fn,
        in_axes=(0, None, None, None, None, None, None, None, None, None),
    )
    return _PMAPPED


def kernel(x, Wq, Wk, Wv, Wproj, bproj, tab_k_v, tab_k_h, tab_v_v, tab_v_h):
    f = _build()
    x = np.asarray(x, dtype=np.float32)
    B, N, E = x.shape
    xs = x.reshape(N_CORES, B // N_CORES, N, E)
    out = f(
        xs,
        jnp.asarray(Wq), jnp.asarray(Wk), jnp.asarray(Wv),
        jnp.asarray(Wproj), jnp.asarray(bproj),
        jnp.asarray(tab_k_v), jnp.asarray(tab_k_h),
        jnp.asarray(tab_v_v), jnp.asarray(tab_v_h),
    )
    return np.asarray(out).reshape(B, N, E).astype(np.float32)
